# revision 5
# baseline (speedup 1.0000x reference)
"""4-bit column-block-quantized linear (ColBlockQuantizedLinear) on 8 Trainium2 NeuronCores.

Reference computation:
    w[n, k] = (nibble(quant_weight)[n, k] - zeros[n]) * scales[n]     n<11008, k<4096
    out[b, s, n] = sum_k inp[b, s, k] * w[n, k]                        inp: [4, 2048, 4096] f32

Strategy (column-parallel, per sharding hint):
  - Shard out_features N=11008 (padded to 11264 = 8*1408) across 8 cores; replicate inp.
  - Host-side layout prep only: transpose/permute inp to k-major bf16, cast packed
    weights int32->uint8 and transpose to [k/2, n] per core, broadcast scale vectors.
  - On-chip per core: unpack nibbles + dequantize into resident SBUF weight tiles
    W[kt] = (q >> {0,4} & 15) * s - s*z  (bf16, k on partitions, n on free dim),
    then a dense bf16 matmul: psum[m, n] += xT[k, m].T @ W[k, n] accumulated over
    32 k-tiles, evicted to f32 and DMA'd to a per-core [8192, 1408] output.
  - Host concatenates per-core outputs along N and drops padding.
"""

import sys

for _p in ("/opt/trn_rl_repo", "/opt/pypackages"):
    if _p not in sys.path:
        sys.path.append(_p)

import numpy as np
import ml_dtypes

import concourse.bass as bass
import concourse.mybir as mybir
import concourse.tile as tile
from concourse import bacc

# Problem constants (hardcoded per harness contract)
B, S, K = 4, 2048, 4096
M = B * S                  # 8192 tokens
N = 11008                  # out features
NCORES = 8
NPC = 1408                 # padded per-core out features (11 * 128); 8*1408 = 11264
KP = K // 2                # packed k rows (2048)
P = 128


def _nchunks(npc):
    return [(i, min(512, npc - i)) for i in range(0, npc, 512)]


def build_nc(m=M, kp=KP, npc=NPC, mg=512):
    """Build the per-core Bass program. m tokens, kp packed-k rows, npc out cols,
    mg tokens per m-group (DMA granule)."""
    ktp = kp // P              # packed k tiles (16 full-size)
    kt_n = 2 * ktp             # unpacked k tiles (32)
    ngroups = m // mg
    mbs = mg // P              # m-blocks per group
    chunks = _nchunks(npc)

    nc = bacc.Bacc("TRN2", target_bir_lowering=False, debug=False)
    xt_d = nc.dram_tensor("xt", [kt_n, P, m], mybir.dt.bfloat16, kind="ExternalInput")
    qwt_d = nc.dram_tensor("qwt", [ktp, P, npc], mybir.dt.uint8, kind="ExternalInput")
    sb_d = nc.dram_tensor("sb", [P, npc], mybir.dt.bfloat16, kind="ExternalInput")
    bb_d = nc.dram_tensor("bb", [P, npc], mybir.dt.bfloat16, kind="ExternalInput")
    out_d = nc.dram_tensor("out", [m, npc], mybir.dt.float32, kind="ExternalOutput")

    with tile.TileContext(nc) as tc:
        with (
            tc.tile_pool(name="const", bufs=1) as const_pool,
            tc.tile_pool(name="stage", bufs=3) as stage_pool,
            tc.tile_pool(name="w", bufs=1) as w_pool,
            tc.tile_pool(name="x", bufs=2) as x_pool,
            tc.tile_pool(name="o", bufs=3) as o_pool,
            tc.tile_pool(name="ps", bufs=2, space="PSUM") as ps_pool,
        ):
            sb = const_pool.tile([P, npc], mybir.dt.bfloat16, tag="sb")
            bb = const_pool.tile([P, npc], mybir.dt.bfloat16, tag="bb")
            nc.sync.dma_start(sb[:], sb_d[:])
            nc.sync.dma_start(bb[:], bb_d[:])

            # Unpack + dequantize weights into resident SBUF tiles.
            # W[kt] for kt in [0, ktp) = low nibbles (even k), [ktp, 2ktp) = high.
            w_tiles = [
                w_pool.tile([P, npc], mybir.dt.bfloat16, name=f"W{kt}", tag=f"W{kt}")
                for kt in range(kt_n)
            ]
            for kt in range(ktp):
                q = stage_pool.tile([P, npc], mybir.dt.uint8, tag="q")
                nc.sync.dma_start(q[:], qwt_d[kt])
                w_lo, w_hi = w_tiles[kt], w_tiles[ktp + kt]
                # nibble extraction (bitwise, u8->u8) then dequant, all on DVE
                lo8 = stage_pool.tile([P, npc], mybir.dt.uint8, tag="lo8")
                hi8 = stage_pool.tile([P, npc], mybir.dt.uint8, tag="hi8")
                nc.vector.tensor_scalar(
                    lo8[:], q[:], 15, None, op0=mybir.AluOpType.bitwise_and
                )
                nc.vector.tensor_scalar(
                    hi8[:], q[:], 4, None, op0=mybir.AluOpType.logical_shift_right
                )
                nc.vector.tensor_tensor(
                    w_lo[:], lo8[:], sb[:], op=mybir.AluOpType.mult
                )
                nc.vector.tensor_sub(w_lo[:], w_lo[:], bb[:])
                nc.vector.tensor_tensor(
                    w_hi[:], hi8[:], sb[:], op=mybir.AluOpType.mult
                )
                nc.vector.tensor_sub(w_hi[:], w_hi[:], bb[:])

            # Main matmul loop: m-groups of `mg` tokens, 128-token m-blocks.
            for g in range(ngroups):
                xg = x_pool.tile([P, kt_n, mg], mybir.dt.bfloat16, tag="xg")
                for kt in range(kt_n):
                    nc.sync.dma_start(
                        xg[:, kt, :], xt_d[kt, :, g * mg:(g + 1) * mg]
                    )
                # consume k-tiles in unpack-completion order (low_t, high_t)
                # so the first m-blocks aren't gated on the whole unpack
                kt_order = [t + h * ktp for t in range(ktp) for h in (0, 1)]
                for mb in range(mbs):
                    ps = ps_pool.tile([P, npc], mybir.dt.float32, tag="ps")
                    for i, kt in enumerate(kt_order):
                        lhsT = xg[:, kt, mb * P:(mb + 1) * P]
                        for (n0, nw) in chunks:
                            nc.tensor.matmul(
                                ps[:, n0:n0 + nw], lhsT, w_tiles[kt][:, n0:n0 + nw],
                                start=(i == 0), stop=(i == kt_n - 1),
                            )
                    ot = o_pool.tile([P, npc], mybir.dt.float32, tag="ot")
                    nc.scalar.copy(ot[:], ps[:])
                    m0 = g * mg + mb * P
                    nc.sync.dma_start(out_d[m0:m0 + P, :], ot[:])

    nc.compile()
    return nc


def prep_inputs(inp, quant_weight, scales, zeros, ncores=NCORES, npc=NPC):
    """Host-side sharding/layout: returns in_maps list for run_bass_kernel_spmd."""
    m = inp.shape[0] * inp.shape[1]
    k = inp.shape[2]
    kp = k // 2
    ktp = kp // P

    x = np.asarray(inp, dtype=np.float32).reshape(m, k)
    x3 = x.reshape(m, kp, 2)
    # xt rows: kt in [0, ktp) -> even k (low nibble), [ktp, 2ktp) -> odd k (high)
    xt_even = np.ascontiguousarray(x3[:, :, 0].T).astype(ml_dtypes.bfloat16)
    xt_odd = np.ascontiguousarray(x3[:, :, 1].T).astype(ml_dtypes.bfloat16)
    xt = np.concatenate(
        [xt_even.reshape(ktp, P, m), xt_odd.reshape(ktp, P, m)], axis=0
    )  # [2*ktp, P, m] bf16

    n = quant_weight.shape[0]
    npad = ncores * npc
    qw8 = np.zeros((npad, kp), np.uint8)
    qw8[:n] = np.asarray(quant_weight).astype(np.uint8)
    s_pad = np.zeros((npad,), np.float32)
    s_pad[:n] = np.asarray(scales, dtype=np.float32).reshape(-1)
    z_pad = np.zeros((npad,), np.float32)
    z_pad[:n] = np.asarray(zeros, dtype=np.float32).reshape(-1)
    b_pad = s_pad * z_pad

    in_maps = []
    for c in range(ncores):
        sl = slice(c * npc, (c + 1) * npc)
        qwt_c = np.ascontiguousarray(qw8[sl].T).reshape(ktp, P, npc)
        sb_c = np.ascontiguousarray(
            np.broadcast_to(s_pad[sl].astype(ml_dtypes.bfloat16), (P, npc))
        )
        bb_c = np.ascontiguousarray(
            np.broadcast_to(b_pad[sl].astype(ml_dtypes.bfloat16), (P, npc))
        )
        in_maps.append({"xt": xt, "qwt": qwt_c, "sb": sb_c, "bb": bb_c})
    return in_maps


_NC_CACHE = {}


def _get_nc():
    if "nc" not in _NC_CACHE:
        _NC_CACHE["nc"] = build_nc()
    return _NC_CACHE["nc"]


def kernel(inp, quant_weight, scales, zeros):
    from concourse.bass_utils import run_bass_kernel_spmd

    nc = _get_nc()
    in_maps = prep_inputs(inp, quant_weight, scales, zeros)
    res = run_bass_kernel_spmd(nc, in_maps, list(range(NCORES)))
    out = np.concatenate([res.results[c]["out"] for c in range(NCORES)], axis=1)
    return np.ascontiguousarray(out[:, :N]).reshape(B, S, N)
